# revision 13
# baseline (speedup 1.0000x reference)
"""Bass/Tile TRN2 kernel for retrieval-KNN MSE distance matrix.

Computes: out = ||t||^2 + ||s@W.T+b||^2 - 2 * t @ (s@W.T+b).T   [N=4096, M=4096]

Sharding (8 cores, output column-parallel):
  core c holds s_rep rows [c*512, (c+1)*512) and computes the full-height
  output block out[:, c*512:(c+1)*512].  t_sq (row norms) is additionally
  sharded: each core computes the norms of its *own* 512-row group (fed
  as the separate per-core input `tg`, keeping the SPMD program
  index-uniform) and an AllGather through a DRAM bounce distributes all
  4096 norms to every core — 8x less gram-matmul work than computing all
  norms on every core.

All matmuls run in fp8 e4m3 with DoubleRow perf mode (two k-subtiles of
128 per instruction, 2x bf16 throughput). Inputs are cast to fp8 on the
host (the same cast the device would otherwise do), which also cuts HBM
traffic 4x vs fp32. Error stays in budget because the distance is
computed consistently in quantized space: out = ||t8 - s'8||^2 exactly
(fp32 accumulation), t8 = fp8(t), s'8 = fp8(-2(s8@W8.T+b))/-2.

Per-core pipeline:
  loads  : s + W chunk 0 (GEMM1 critical path), tg, ident, b, W chunks
           1-3, then the 8 t row-groups; everything is SBUF-resident.
  tgram  : own-group row norms via fp8 gram matmuls (diag extracted with
           DVE mask-mul + reduce; tensor_tensor_reduce hangs TRN2 hw),
           AllGather on the gpsimd queue -> tsq_full [128, 8, 4].
  GEMM1  : sproj_m2[d,c] = -2*(W8.T @ s8 + b) in fp8 (ACT copyback);
           squares (DVE) and s_sq ones-matmuls interleaved, staggered 3
           behind so the PE never waits on them.
  s_sq   : [1,512] row scaled 0.25 (ACT) -> K=1 ones matmul broadcast
           to [128,512] psum -> fp16 SBUF copy (ACT).
  main   : psum = t8_j.T @ sproj_m2 (= -2*cross), 6 DoubleRow matmuls.
  epilog : ACT tmp16 = psum + t_sq (bias add), Pool ob = tmp16 + s_sq;
           fp16 stores batched 4 row-chunks per DMA.
Output fp16 (ulp ~1 at |out|~4e3), upcast to fp32 on host.
"""

import numpy as np
import ml_dtypes

import concourse.bacc as bacc
import concourse.bass as bass
import concourse.mybir as mybir
import concourse.tile as tile
from concourse.alu_op_type import AluOpType
from concourse.bass_utils import run_bass_kernel_spmd

N = 4096          # t_rep rows
M = 4096          # s_rep rows
D = 1536          # feature dim
NCORES = 8
MC = M // NCORES  # 512: output columns per core
KC = D // 128     # 12:  contraction chunks of 128
NJ = N // 128     # 32:  output row chunks per core
NG = NJ // 4      # 8:   512-row groups
WCH = 4           # W column chunks
WCOLS = D // WCH  # 384 cols per W chunk

FP32 = mybir.dt.float32
FP16 = mybir.dt.float16
BF16 = mybir.dt.bfloat16
FP8 = mybir.dt.float8e4
AF = mybir.ActivationFunctionType
DR = mybir.MatmulPerfMode.DoubleRow

NP_FP8 = ml_dtypes.float8_e4m3  # matches mybir.dt.np(dt.float8e4)

N_WARM = 24       # PE p-state warmup matmuls
TSQ_COLLECTIVE = True


def build_nc():
    nc = bacc.Bacc("TRN2", target_bir_lowering=False, num_devices=NCORES)

    # host layouts are partition-major so each load is one big DMA
    t_in = nc.dram_tensor("t", [NG, 128, KC, 512], FP8, kind="ExternalInput").ap()
    tg_in = nc.dram_tensor("tg", [128, KC, 512], FP8, kind="ExternalInput").ap()
    s_in = nc.dram_tensor("s", [128, KC, MC], FP8, kind="ExternalInput").ap()
    w_in = nc.dram_tensor("w", [WCH, 128, KC, WCOLS], FP8,
                          kind="ExternalInput").ap()
    b_in = nc.dram_tensor("bneg2", [128, KC], FP32, kind="ExternalInput").ap()
    id_in = nc.dram_tensor("ident", [128, 128], FP32, kind="ExternalInput").ap()
    out = nc.dram_tensor("out", [NG, 128, 4, MC], FP16, kind="ExternalOutput").ap()

    with tile.TileContext(nc) as tc:
        with (
            tc.tile_pool(name="const", bufs=1) as const_pool,
            tc.tile_pool(name="wsb", bufs=WCH) as w_pool,
            tc.tile_pool(name="ssb", bufs=1) as s_pool,
            tc.tile_pool(name="sproj", bufs=1) as sproj_pool,
            tc.tile_pool(name="tsb", bufs=NG) as t_pool,
            tc.tile_pool(name="tgsb", bufs=1) as tg_pool,
            tc.tile_pool(name="sq", bufs=KC) as sq_pool,
            tc.tile_pool(name="tsq", bufs=4) as tsq_pool,
            tc.tile_pool(name="scr", bufs=2) as scr_pool,
            tc.tile_pool(name="osb", bufs=3) as out_pool,
            tc.tile_pool(name="tmp", bufs=3) as tmp_pool,
            tc.tile_pool(name="dram", bufs=2, space="DRAM") as dram_pool,
            tc.tile_pool(name="psum_main", bufs=4, space="PSUM") as psum_main,
            tc.tile_pool(name="psum_gram", bufs=2, space="PSUM") as psum_gram,
            tc.tile_pool(name="psum_ssq", bufs=1, space="PSUM") as psum_ssq,
        ):
            # ---- input loads: GEMM1 operands first (critical path) ----
            ssb = s_pool.tile([128, KC, MC], FP8)
            nc.sync.dma_start(out=ssb[:], in_=s_in[:])
            wsb = []
            for c in range(WCH):
                wt = w_pool.tile([128, KC, WCOLS], FP8, name="wsb")
                nc.sync.dma_start(out=wt[:], in_=w_in[c])
                wsb.append(wt)
                if c == 0:
                    tgsb = tg_pool.tile([128, KC, 512], FP8)
                    nc.sync.dma_start(out=tgsb[:], in_=tg_in[:])
                    ident = const_pool.tile([128, 128], FP32)
                    nc.sync.dma_start(out=ident[:], in_=id_in[:])
                    bsb = const_pool.tile([128, KC], FP32)
                    nc.sync.dma_start(out=bsb[:], in_=b_in[:])
            tsb = []
            for g in range(NG):
                tt = t_pool.tile([128, KC, 512], FP8, name="tsb")
                nc.sync.dma_start(out=tt[:], in_=t_in[g])
                tsb.append(tt)

            ones_k = const_pool.tile([128, 1], BF16)
            nc.vector.memset(ones_k[:], 1.0)
            ones_1 = const_pool.tile([1, 128], FP16)
            nc.vector.memset(ones_1[:], 1.0)

            # ---- PE p-state warmup while initial DMAs stream ----
            warm = const_pool.tile([128, 2, 512], FP8, name="warm")
            nc.vector.memset(warm[:], 0.5)
            pw = psum_main.tile([128, MC], FP32, name="pw", tag="mm")
            for i in range(N_WARM):
                nc.tensor.matmul(pw[:], lhsT=warm[:, :, 0:128], rhs=warm[:],
                                 start=(i == 0), stop=(i == N_WARM - 1),
                                 perf_mode=DR)

            # ---- own-group t_sq -> AllGather (issued around GEMM1 jc=2) ----
            tsq_own = const_pool.tile([128, 4], FP32, name="tsq_own")
            tsq_full = const_pool.tile([128, NG, 4], FP32, name="tsq_full")
            cc_in = dram_pool.tile([128, 4], FP32)
            cc_out = dram_pool.tile([NG, 128, 4], FP32)

            def tgram(jj):
                gram = psum_gram.tile([128, 128], FP32, name="psum_gram")
                rsl = slice(jj * 128, (jj + 1) * 128)
                for a in range(KC // 2):
                    ksl = slice(2 * a, 2 * a + 2)
                    nc.tensor.matmul(
                        gram[:],
                        lhsT=tgsb[:, ksl, rsl],
                        rhs=tgsb[:, ksl, rsl],
                        start=(a == 0),
                        stop=(a == KC // 2 - 1),
                        perf_mode=DR,
                    )
                scr = scr_pool.tile([128, 128], FP32, name="scr")
                nc.vector.tensor_mul(scr[:], gram[:], ident[:])
                nc.vector.reduce_sum(tsq_own[:, jj:jj + 1], scr[:],
                                     axis=mybir.AxisListType.X)

            def tsq_gather():
                nc.gpsimd.dma_start(out=cc_in[:], in_=tsq_own[:])
                nc.gpsimd.collective_compute(
                    "AllGather",
                    mybir.AluOpType.bypass,
                    replica_groups=[list(range(NCORES))],
                    ins=[cc_in.opt()],
                    outs=[cc_out.opt()],
                )
                for k in range(NG):
                    nc.gpsimd.dma_start(out=tsq_full[:, k, :], in_=cc_out[k])

            # ---- GEMM1 + interleaved s_sq reduction + tgram ----
            sproj = sproj_pool.tile([128, KC, MC], FP8)
            sq_tiles = []
            psum_sq = psum_ssq.tile([1, MC], FP32, name="psum_ssq")

            def ssq_mm(jc):
                nc.tensor.matmul(
                    psum_sq[:], lhsT=ones_k[:], rhs=sq_tiles[jc][:],
                    start=(jc == 0), stop=(jc == KC - 1),
                )

            for jc in range(KC):
                ps = psum_main.tile([128, MC], FP32, name="psum_g1", tag="mm")
                wt = wsb[jc // (KC // WCH)]
                col = (jc % (KC // WCH)) * 128
                for a in range(KC // 2):
                    nc.tensor.matmul(
                        ps[:],
                        lhsT=wt[:, 2 * a:2 * a + 2, col:col + 128],
                        rhs=ssb[:, 2 * a:2 * a + 2, :],
                        start=(a == 0),
                        stop=(a == KC // 2 - 1),
                        perf_mode=DR,
                    )
                if jc == 2 and TSQ_COLLECTIVE:
                    # tg has landed by now; PE fills the w1/w2 load gap
                    for jj in range(4):
                        tgram(jj)
                    tsq_gather()
                if jc >= 3:
                    ssq_mm(jc - 3)  # staggered so the PE never waits
                nc.scalar.activation(sproj[:, jc, :], ps[:], AF.Identity,
                                     bias=bsb[:, jc:jc + 1], scale=-2.0)
                sq = sq_pool.tile([128, MC], BF16, name="sq")
                nc.vector.tensor_mul(sq[:], sproj[:, jc, :], sproj[:, jc, :])
                sq_tiles.append(sq)
            for jc in range(KC - 3, KC):
                ssq_mm(jc)

            # s_sq row (0.25x) then K=1 ones-matmul broadcast to [128, MC]
            ssq_row = const_pool.tile([1, MC], FP16, name="ssq_row")
            nc.scalar.activation(ssq_row[:], psum_sq[:], AF.Identity,
                                 scale=0.25)
            ssq_bc16 = const_pool.tile([128, MC], FP16, name="ssq_bc16")

            # ---- main loop over 32 row-chunks ----
            first = True
            for g in range(NG):
                ob = out_pool.tile([128, 4, MC], FP16, name="osb")
                for jj in range(4):
                    ps = psum_main.tile([128, MC], FP32, name="psum_mm", tag="mm")
                    rsl = slice(jj * 128, (jj + 1) * 128)
                    for a in range(KC // 2):
                        ksl = slice(2 * a, 2 * a + 2)
                        nc.tensor.matmul(
                            ps[:],
                            lhsT=tsb[g][:, ksl, rsl],
                            rhs=sproj[:, ksl, :],
                            start=(a == 0),
                            stop=(a == KC // 2 - 1),
                            perf_mode=DR,
                        )
                    if first:
                        # broadcast matmul placed after j0's matmuls so the
                        # PE doesn't stall waiting for the s_sq row
                        bps = psum_ssq.tile([128, MC], FP32, name="psum_bc",
                                            bufs=1)
                        nc.tensor.matmul(bps[:], lhsT=ones_1[:], rhs=ssq_row[:],
                                         start=True, stop=True)
                        nc.scalar.activation(ssq_bc16[:], bps[:], AF.Identity)
                        first = False
                    # out = (psum + t_sq) + s_sq   (psum = -2*cross)
                    tmp = tmp_pool.tile([128, MC], FP16, name="tmp")
                    nc.scalar.activation(tmp[:], ps[:], AF.Identity,
                                         bias=tsq_full[:, g, jj:jj + 1],
                                         scale=1.0)
                    nc.gpsimd.tensor_add(ob[:, jj, :], tmp[:], ssq_bc16[:])
                nc.sync.dma_start(out=out[g], in_=ob[:])

    nc.compile()
    return nc


_NC_CACHE = None


def _get_nc():
    global _NC_CACHE
    if _NC_CACHE is None:
        _NC_CACHE = build_nc()
    return _NC_CACHE


def stage_inputs(t_rep, s_rep, W, b):
    """Host-side layout + precision staging -> per-core input maps."""
    t_rep = np.asarray(t_rep, dtype=np.float32)
    s_rep = np.asarray(s_rep, dtype=np.float32)
    W = np.asarray(W, dtype=np.float32)
    b = np.asarray(b, dtype=np.float32)

    # t8[g, p, k, r] = t[g*512 + r, k*128 + p]
    t8 = np.ascontiguousarray(
        t_rep.reshape(NG, 512, KC, 128).transpose(0, 3, 2, 1)
    ).astype(NP_FP8)
    # w8[c, p, k, m] = W[c*384 + m, k*128 + p]
    w8 = np.ascontiguousarray(
        W.reshape(WCH, WCOLS, KC, 128).transpose(0, 3, 2, 1)
    ).astype(NP_FP8)
    # bneg2[p, k] = -2*b[k*128+p]
    bneg2 = np.ascontiguousarray((-2.0 * b).reshape(KC, 128).T)
    ident = np.eye(128, dtype=np.float32)

    in_maps = []
    for c in range(NCORES):
        s_slice = s_rep[c * MC:(c + 1) * MC]  # [512, D]
        # s8[p, k, r] = s_slice[r, k*128 + p]
        s8 = np.ascontiguousarray(
            s_slice.reshape(MC, KC, 128).transpose(2, 1, 0)
        ).astype(NP_FP8)
        in_maps.append({"t": t8, "tg": np.ascontiguousarray(t8[c]), "s": s8,
                        "w": w8, "bneg2": bneg2, "ident": ident})
    return in_maps


def run_spmd(in_maps, **kwargs):
    nc = _get_nc()
    return run_bass_kernel_spmd(nc, in_maps, core_ids=list(range(NCORES)), **kwargs)


def gather_output(results):
    cols = []
    for c in range(NCORES):
        o = np.asarray(results[c]["out"])  # [NG, 128, 4, MC] fp16
        cols.append(o.transpose(0, 2, 1, 3).reshape(N, MC).astype(np.float32))
    return np.concatenate(cols, axis=1)


def kernel(t_rep, s_rep, W, b):
    in_maps = stage_inputs(t_rep, s_rep, W, b)
    res = run_spmd(in_maps)
    return gather_output(res.results)


# revision 16
# speedup vs baseline: 1.4058x; 1.4058x over previous
"""Bass/Tile TRN2 kernel for retrieval-KNN MSE distance matrix.

Computes: out = ||t||^2 + ||s@W.T+b||^2 - 2 * t @ (s@W.T+b).T   [N=4096, M=4096]

Sharding (8 cores, output column-parallel, no collectives):
  core c holds s_rep rows [c*512, (c+1)*512) and computes the full-height
  output block out[:, c*512:(c+1)*512].

All matmuls run in fp8 e4m3 with DoubleRow perf mode (two k-subtiles of
128 per instruction, 2x bf16 throughput). Inputs are cast to fp8 on the
host (the same cast the device would otherwise do), which also cuts HBM
traffic 4x vs fp32. Error stays in budget because the distance is
computed consistently in quantized space: out = ||t8 - s'8||^2 exactly
(fp32 accumulation), t8 = fp8(t), s'8 = fp8(-2(s8@W8.T+b))/-2.

Per-core pipeline:
  loads  : s + W chunk 0 (GEMM1 critical path), ident, b, W chunks 1-3,
           then the 8 t row-groups; everything is SBUF-resident once.
  GEMM1  : sproj_m2[d,c] = -2*(W8.T @ s8 + b) in fp8 (ACT copyback);
           squares (DVE) and s_sq ones-matmuls interleaved, staggered 3
           behind so the PE never waits on them.
  s_sq   : [1,512] row scaled 0.25 (ACT) -> K=1 ones matmul broadcast
           to [128,512] psum -> fp16 SBUF copy (ACT).
  main   : psum = t8_j.T @ sproj_m2 (= -2*cross) + gram = t8_j.T @ t8_j,
           6 DoubleRow matmuls each (gram first: shares lhsT with main);
           t_sq = diag(gram) via DVE mask-mul + reduce
           (tensor_tensor_reduce hangs TRN2 hw; a cross-core t_sq
           AllGather was tried and loses: collective_compute implies a
           ~45us all-engine barrier on this runtime).
  epilog : ACT tmp16 = psum + t_sq (bias add), Pool ob = tmp16 + s_sq;
           fp16 stores batched 4 row-chunks per DMA.
Output fp16 (ulp ~1 at |out|~4e3), upcast to fp32 on host.
"""

import numpy as np
import ml_dtypes

import concourse.bacc as bacc
import concourse.bass as bass
import concourse.mybir as mybir
import concourse.tile as tile
from concourse.alu_op_type import AluOpType
from concourse.bass_utils import run_bass_kernel_spmd

N = 4096          # t_rep rows
M = 4096          # s_rep rows
D = 1536          # feature dim
NCORES = 8
MC = M // NCORES  # 512: output columns per core
KC = D // 128     # 12:  contraction chunks of 128
NJ = N // 128     # 32:  output row chunks per core
NG = NJ // 4      # 8:   512-row groups
WCH = 4           # W column chunks
WCOLS = D // WCH  # 384 cols per W chunk

FP32 = mybir.dt.float32
FP16 = mybir.dt.float16
BF16 = mybir.dt.bfloat16
FP8 = mybir.dt.float8e4
AF = mybir.ActivationFunctionType
DR = mybir.MatmulPerfMode.DoubleRow

NP_FP8 = ml_dtypes.float8_e4m3  # matches mybir.dt.np(dt.float8e4)

N_WARM = 40       # PE p-state warmup matmuls (bridges idle gap to GEMM1)


def build_nc():
    nc = bacc.Bacc("TRN2", target_bir_lowering=False, num_devices=NCORES)

    # host layouts are partition-major so each load is one big DMA
    t_in = nc.dram_tensor("t", [NG, 128, KC, 512], FP8, kind="ExternalInput").ap()
    s_in = nc.dram_tensor("s", [128, KC, MC], FP8, kind="ExternalInput").ap()
    w_in = nc.dram_tensor("w", [WCH, 128, KC, WCOLS], FP8,
                          kind="ExternalInput").ap()
    b_in = nc.dram_tensor("bneg2", [128, KC], FP32, kind="ExternalInput").ap()
    id_in = nc.dram_tensor("ident", [128, 128], FP32, kind="ExternalInput").ap()
    out = nc.dram_tensor("out", [NG, 128, 4, MC], FP16, kind="ExternalOutput").ap()

    with tile.TileContext(nc) as tc:
        with (
            tc.tile_pool(name="const", bufs=1) as const_pool,
            tc.tile_pool(name="wsb", bufs=WCH) as w_pool,
            tc.tile_pool(name="ssb", bufs=1) as s_pool,
            tc.tile_pool(name="sproj", bufs=1) as sproj_pool,
            tc.tile_pool(name="tsb", bufs=NG) as t_pool,
            tc.tile_pool(name="sq", bufs=KC) as sq_pool,
            tc.tile_pool(name="tsq", bufs=4) as tsq_pool,
            tc.tile_pool(name="scr", bufs=2) as scr_pool,
            tc.tile_pool(name="osb", bufs=3) as out_pool,
            tc.tile_pool(name="tmp", bufs=3) as tmp_pool,
            tc.tile_pool(name="psum_main", bufs=5, space="PSUM") as psum_main,
            tc.tile_pool(name="psum_gram", bufs=2, space="PSUM") as psum_gram,
            tc.tile_pool(name="psum_ssq", bufs=1, space="PSUM") as psum_ssq,
        ):
            # ---- input loads: GEMM1 operands first (critical path) ----
            ssb = s_pool.tile([128, KC, MC], FP8)
            nc.sync.dma_start(out=ssb[:], in_=s_in[:])
            wsb = []
            for c in range(WCH):
                wt = w_pool.tile([128, KC, WCOLS], FP8, name="wsb")
                nc.sync.dma_start(out=wt[:], in_=w_in[c])
                wsb.append(wt)
                if c == 0:
                    ident = const_pool.tile([128, 128], FP32)
                    nc.sync.dma_start(out=ident[:], in_=id_in[:])
                    bsb = const_pool.tile([128, KC], FP32)
                    nc.sync.dma_start(out=bsb[:], in_=b_in[:])
            tsb = []
            for g in range(NG):
                tt = t_pool.tile([128, KC, 512], FP8, name="tsb")
                nc.sync.dma_start(out=tt[:], in_=t_in[g])
                tsb.append(tt)

            ones_k = const_pool.tile([128, 1], BF16)
            nc.vector.memset(ones_k[:], 1.0)
            ones_1 = const_pool.tile([1, 128], FP16)
            nc.vector.memset(ones_1[:], 1.0)

            # ---- PE p-state warmup while initial DMAs stream ----
            warm = const_pool.tile([128, 2, 512], FP8, name="warm")
            nc.vector.memset(warm[:], 0.5)
            pw = psum_main.tile([128, MC], FP32, name="pw", tag="mm")
            for i in range(N_WARM):
                nc.tensor.matmul(pw[:], lhsT=warm[:, :, 0:128], rhs=warm[:],
                                 start=(i == 0), stop=(i == N_WARM - 1),
                                 perf_mode=DR)

            # ---- GEMM1 + interleaved s_sq reduction ----
            sproj = sproj_pool.tile([128, KC, MC], FP8)
            sq_tiles = []
            psum_sq = psum_ssq.tile([1, MC], FP32, name="psum_ssq", tag="ssq")

            def ssq_mm(jc):
                nc.tensor.matmul(
                    psum_sq[:], lhsT=ones_k[:], rhs=sq_tiles[jc][:],
                    start=(jc == 0), stop=(jc == KC - 1),
                )

            for jc in range(KC):
                ps = psum_main.tile([128, MC], FP32, name="psum_g1", tag="mm")
                wt = wsb[jc // (KC // WCH)]
                col = (jc % (KC // WCH)) * 128
                for a in range(KC // 2):
                    nc.tensor.matmul(
                        ps[:],
                        lhsT=wt[:, 2 * a:2 * a + 2, col:col + 128],
                        rhs=ssb[:, 2 * a:2 * a + 2, :],
                        start=(a == 0),
                        stop=(a == KC // 2 - 1),
                        perf_mode=DR,
                    )
                if jc >= 3:
                    ssq_mm(jc - 3)  # staggered so the PE never waits
                nc.scalar.activation(sproj[:, jc, :], ps[:], AF.Identity,
                                     bias=bsb[:, jc:jc + 1], scale=-2.0)
                sq = sq_pool.tile([128, MC], BF16, name="sq")
                nc.vector.tensor_mul(sq[:], sproj[:, jc, :], sproj[:, jc, :])
                sq_tiles.append(sq)
            for jc in range(KC - 3, KC):
                ssq_mm(jc)

            # s_sq row (0.25x) then K=1 ones-matmul broadcast to [128, MC]
            ssq_row = const_pool.tile([1, MC], FP16, name="ssq_row")
            nc.scalar.activation(ssq_row[:], psum_sq[:], AF.Identity,
                                 scale=0.25)
            ssq_bc16 = const_pool.tile([128, MC], FP16, name="ssq_bc16")

            # ---- main loop over 32 row-chunks ----
            first = True
            for g in range(NG):
                ob = out_pool.tile([128, 4, MC], FP16, name="osb")
                for jj in range(4):
                    ps = psum_main.tile([128, MC], FP32, name="psum_mm", tag="mm")
                    gram = psum_gram.tile([128, 128], FP32, name="psum_gram")
                    rsl = slice(jj * 128, (jj + 1) * 128)
                    for a in range(KC // 2):
                        ksl = slice(2 * a, 2 * a + 2)
                        # gram first: it shares lhsT with the main matmul
                        nc.tensor.matmul(
                            gram[:],
                            lhsT=tsb[g][:, ksl, rsl],
                            rhs=tsb[g][:, ksl, rsl],
                            start=(a == 0),
                            stop=(a == KC // 2 - 1),
                            perf_mode=DR,
                        )
                        nc.tensor.matmul(
                            ps[:],
                            lhsT=tsb[g][:, ksl, rsl],
                            rhs=sproj[:, ksl, :],
                            start=(a == 0),
                            stop=(a == KC // 2 - 1),
                            perf_mode=DR,
                        )
                    if first:
                        first = False
                        # K=1 broadcast matmul after j0's matmuls (the j0
                        # epilogue below must see the write in program
                        # order); shares the ssq psum bank (freed by
                        # ssq_row's read)
                        bps = psum_ssq.tile([128, MC], FP32,
                                            name="psum_bc", tag="ssq")
                        nc.tensor.matmul(bps[:], lhsT=ones_1[:],
                                         rhs=ssq_row[:],
                                         start=True, stop=True)
                        nc.scalar.activation(ssq_bc16[:], bps[:],
                                             AF.Identity)
                    # t_sq[p] = sum_f gram[p,f] * I[p,f]
                    tsq = tsq_pool.tile([128, 1], FP32, name="tsq")
                    scr = scr_pool.tile([128, 128], FP32, name="scr")
                    nc.vector.tensor_mul(scr[:], gram[:], ident[:])
                    nc.vector.reduce_sum(tsq[:], scr[:],
                                         axis=mybir.AxisListType.X)
                    # out = (psum + t_sq) + s_sq   (psum = -2*cross)
                    tmp = tmp_pool.tile([128, MC], FP16, name="tmp")
                    nc.scalar.activation(tmp[:], ps[:], AF.Identity,
                                         bias=tsq[:], scale=1.0)
                    nc.gpsimd.tensor_add(ob[:, jj, :], tmp[:], ssq_bc16[:])
                nc.sync.dma_start(out=out[g], in_=ob[:])

    nc.compile()
    return nc


_NC_CACHE = None


def _get_nc():
    global _NC_CACHE
    if _NC_CACHE is None:
        _NC_CACHE = build_nc()
    return _NC_CACHE


def stage_inputs(t_rep, s_rep, W, b):
    """Host-side layout + precision staging -> per-core input maps."""
    t_rep = np.asarray(t_rep, dtype=np.float32)
    s_rep = np.asarray(s_rep, dtype=np.float32)
    W = np.asarray(W, dtype=np.float32)
    b = np.asarray(b, dtype=np.float32)

    # t8[g, p, k, r] = t[g*512 + r, k*128 + p]
    t8 = np.ascontiguousarray(
        t_rep.reshape(NG, 512, KC, 128).transpose(0, 3, 2, 1)
    ).astype(NP_FP8)
    # w8[c, p, k, m] = W[c*384 + m, k*128 + p]
    w8 = np.ascontiguousarray(
        W.reshape(WCH, WCOLS, KC, 128).transpose(0, 3, 2, 1)
    ).astype(NP_FP8)
    # bneg2[p, k] = -2*b[k*128+p]
    bneg2 = np.ascontiguousarray((-2.0 * b).reshape(KC, 128).T)
    ident = np.eye(128, dtype=np.float32)

    in_maps = []
    for c in range(NCORES):
        s_slice = s_rep[c * MC:(c + 1) * MC]  # [512, D]
        # s8[p, k, r] = s_slice[r, k*128 + p]
        s8 = np.ascontiguousarray(
            s_slice.reshape(MC, KC, 128).transpose(2, 1, 0)
        ).astype(NP_FP8)
        in_maps.append({"t": t8, "s": s8, "w": w8, "bneg2": bneg2,
                        "ident": ident})
    return in_maps


def run_spmd(in_maps, **kwargs):
    nc = _get_nc()
    return run_bass_kernel_spmd(nc, in_maps, core_ids=list(range(NCORES)), **kwargs)


def gather_output(results):
    cols = []
    for c in range(NCORES):
        o = np.asarray(results[c]["out"])  # [NG, 128, 4, MC] fp16
        cols.append(o.transpose(0, 2, 1, 3).reshape(N, MC).astype(np.float32))
    return np.concatenate(cols, axis=1)


def kernel(t_rep, s_rep, W, b):
    in_maps = stage_inputs(t_rep, s_rep, W, b)
    res = run_spmd(in_maps)
    return gather_output(res.results)


# revision 17
# speedup vs baseline: 1.4331x; 1.0194x over previous
"""Bass/Tile TRN2 kernel for retrieval-KNN MSE distance matrix.

Computes: out = ||t||^2 + ||s@W.T+b||^2 - 2 * t @ (s@W.T+b).T   [N=4096, M=4096]

Sharding (8 cores, output column-parallel, no collectives):
  core c holds s_rep rows [c*512, (c+1)*512) and computes the full-height
  output block out[:, c*512:(c+1)*512].

All matmuls run in fp8 e4m3 with DoubleRow perf mode (two k-subtiles of
128 per instruction, 2x bf16 throughput). Inputs are cast to fp8 on the
host (the same cast the device would otherwise do), which also cuts HBM
traffic 4x vs fp32. Error stays in budget because the distance is
computed consistently in quantized space: out = ||t8 - s'8||^2 exactly
(fp32 accumulation), t8 = fp8(t), s'8 = fp8(-2(s8@W8.T+b))/-2.

Per-core pipeline:
  loads  : s + W chunk 0 (GEMM1 critical path), ident, b, W chunks 1-3,
           then the 8 t row-groups; everything is SBUF-resident once.
  GEMM1  : sproj_m2[d,c] = -2*(W8.T @ s8 + b) in fp8 (ACT copyback);
           squares (DVE) and s_sq ones-matmuls interleaved, staggered 3
           behind so the PE never waits on them.
  s_sq   : [1,512] row scaled 0.25 (ACT) -> K=1 ones matmul broadcast
           to [128,512] psum -> fp16 SBUF copy (ACT).
  main   : psum = t8_j.T @ sproj_m2 (= -2*cross) + gram = t8_j.T @ t8_j,
           6 DoubleRow matmuls each (gram first: shares lhsT with main);
           t_sq = diag(gram) via DVE mask-mul + reduce
           (tensor_tensor_reduce hangs TRN2 hw; a cross-core t_sq
           AllGather was tried and loses: collective_compute implies a
           ~45us all-engine barrier on this runtime).
  epilog : ACT tmp16 = psum + t_sq (bias add), Pool ob = tmp16 + s_sq;
           fp16 stores batched 4 row-chunks per DMA.
Output fp16 (ulp ~1 at |out|~4e3), upcast to fp32 on host.
"""

import numpy as np
import ml_dtypes

import concourse.bacc as bacc
import concourse.bass as bass
import concourse.mybir as mybir
import concourse.tile as tile
from concourse.alu_op_type import AluOpType
from concourse.bass_utils import run_bass_kernel_spmd

N = 4096          # t_rep rows
M = 4096          # s_rep rows
D = 1536          # feature dim
NCORES = 8
MC = M // NCORES  # 512: output columns per core
KC = D // 128     # 12:  contraction chunks of 128
NJ = N // 128     # 32:  output row chunks per core
NG = NJ // 4      # 8:   512-row groups
WCH = 4           # W column chunks
WCOLS = D // WCH  # 384 cols per W chunk

FP32 = mybir.dt.float32
FP16 = mybir.dt.float16
BF16 = mybir.dt.bfloat16
FP8 = mybir.dt.float8e4
AF = mybir.ActivationFunctionType
DR = mybir.MatmulPerfMode.DoubleRow

NP_FP8 = ml_dtypes.float8_e4m3  # matches mybir.dt.np(dt.float8e4)

N_WARM = 24       # PE p-state warmup matmuls


def build_nc():
    nc = bacc.Bacc("TRN2", target_bir_lowering=False, num_devices=NCORES)

    # host layouts are partition-major so each load is one big DMA
    t_in = nc.dram_tensor("t", [NG, 128, KC, 512], FP8, kind="ExternalInput").ap()
    s_in = nc.dram_tensor("s", [128, KC, MC], FP8, kind="ExternalInput").ap()
    w_in = nc.dram_tensor("w", [WCH, 128, KC, WCOLS], FP8,
                          kind="ExternalInput").ap()
    b_in = nc.dram_tensor("bneg2", [128, KC], FP32, kind="ExternalInput").ap()
    id_in = nc.dram_tensor("ident", [128, 128], FP32, kind="ExternalInput").ap()
    out = nc.dram_tensor("out", [NG, 128, 4, MC], FP16, kind="ExternalOutput").ap()

    with tile.TileContext(nc) as tc:
        with (
            tc.tile_pool(name="const", bufs=1) as const_pool,
            tc.tile_pool(name="wsb", bufs=WCH) as w_pool,
            tc.tile_pool(name="ssb", bufs=1) as s_pool,
            tc.tile_pool(name="sproj", bufs=1) as sproj_pool,
            tc.tile_pool(name="tsb", bufs=NG) as t_pool,
            tc.tile_pool(name="sq", bufs=KC) as sq_pool,
            tc.tile_pool(name="tsq", bufs=4) as tsq_pool,
            tc.tile_pool(name="scr", bufs=2) as scr_pool,
            tc.tile_pool(name="osb", bufs=3) as out_pool,
            tc.tile_pool(name="tmp", bufs=3) as tmp_pool,
            tc.tile_pool(name="psum_main", bufs=4, space="PSUM") as psum_main,
            tc.tile_pool(name="psum_gram", bufs=2, space="PSUM") as psum_gram,
            tc.tile_pool(name="psum_ssq", bufs=1, space="PSUM") as psum_ssq,
        ):
            # ---- input loads: GEMM1 operands first (critical path) ----
            ssb = s_pool.tile([128, KC, MC], FP8)
            nc.sync.dma_start(out=ssb[:], in_=s_in[:])
            wsb = []
            for c in range(WCH):
                wt = w_pool.tile([128, KC, WCOLS], FP8, name="wsb")
                nc.sync.dma_start(out=wt[:], in_=w_in[c])
                wsb.append(wt)
            bsb = const_pool.tile([128, KC], FP32)
            nc.sync.dma_start(out=bsb[:], in_=b_in[:])
            ident = const_pool.tile([128, 128], FP32)
            nc.sync.dma_start(out=ident[:], in_=id_in[:])
            tsb = []
            for g in range(NG):
                tt = t_pool.tile([128, KC, 512], FP8, name="tsb")
                nc.sync.dma_start(out=tt[:], in_=t_in[g])
                tsb.append(tt)

            ones_k = const_pool.tile([128, 1], BF16)
            nc.vector.memset(ones_k[:], 1.0)
            ones_1 = const_pool.tile([1, 128], FP16)
            nc.vector.memset(ones_1[:], 1.0)

            # ---- PE p-state warmup while initial DMAs stream ----
            warm = const_pool.tile([128, 2, 512], FP8, name="warm")
            nc.vector.memset(warm[:], 0.5)
            pw = psum_main.tile([128, MC], FP32, name="pw", tag="mm")
            for i in range(N_WARM):
                nc.tensor.matmul(pw[:], lhsT=warm[:, :, 0:128], rhs=warm[:],
                                 start=(i == 0), stop=(i == N_WARM - 1),
                                 perf_mode=DR)

            # ---- GEMM1 + interleaved s_sq reduction ----
            sproj = sproj_pool.tile([128, KC, MC], FP8)
            sq_tiles = []
            psum_sq = psum_ssq.tile([1, MC], FP32, name="psum_ssq")

            def ssq_mm(jc):
                nc.tensor.matmul(
                    psum_sq[:], lhsT=ones_k[:], rhs=sq_tiles[jc][:],
                    start=(jc == 0), stop=(jc == KC - 1),
                )

            for jc in range(KC):
                ps = psum_main.tile([128, MC], FP32, name="psum_g1", tag="mm")
                wt = wsb[jc // (KC // WCH)]
                col = (jc % (KC // WCH)) * 128
                for a in range(KC // 2):
                    nc.tensor.matmul(
                        ps[:],
                        lhsT=wt[:, 2 * a:2 * a + 2, col:col + 128],
                        rhs=ssb[:, 2 * a:2 * a + 2, :],
                        start=(a == 0),
                        stop=(a == KC // 2 - 1),
                        perf_mode=DR,
                    )
                if jc >= 3:
                    ssq_mm(jc - 3)  # staggered so the PE never waits
                nc.scalar.activation(sproj[:, jc, :], ps[:], AF.Identity,
                                     bias=bsb[:, jc:jc + 1], scale=-2.0)
                sq = sq_pool.tile([128, MC], BF16, name="sq")
                nc.vector.tensor_mul(sq[:], sproj[:, jc, :], sproj[:, jc, :])
                sq_tiles.append(sq)
            for jc in range(KC - 3, KC):
                ssq_mm(jc)

            # s_sq row (0.25x) then K=1 ones-matmul broadcast to [128, MC]
            ssq_row = const_pool.tile([1, MC], FP16, name="ssq_row")
            nc.scalar.activation(ssq_row[:], psum_sq[:], AF.Identity,
                                 scale=0.25)
            ssq_bc16 = const_pool.tile([128, MC], FP16, name="ssq_bc16")

            # ---- main loop over 32 row-chunks ----
            first = True
            for g in range(NG):
                ob = out_pool.tile([128, 4, MC], FP16, name="osb")
                for jj in range(4):
                    ps = psum_main.tile([128, MC], FP32, name="psum_mm", tag="mm")
                    gram = psum_gram.tile([128, 128], FP32, name="psum_gram")
                    rsl = slice(jj * 128, (jj + 1) * 128)
                    for a in range(KC // 2):
                        ksl = slice(2 * a, 2 * a + 2)
                        # gram first: it shares lhsT with the main matmul
                        nc.tensor.matmul(
                            gram[:],
                            lhsT=tsb[g][:, ksl, rsl],
                            rhs=tsb[g][:, ksl, rsl],
                            start=(a == 0),
                            stop=(a == KC // 2 - 1),
                            perf_mode=DR,
                        )
                        nc.tensor.matmul(
                            ps[:],
                            lhsT=tsb[g][:, ksl, rsl],
                            rhs=sproj[:, ksl, :],
                            start=(a == 0),
                            stop=(a == KC // 2 - 1),
                            perf_mode=DR,
                        )
                    if first:
                        first = False
                        # K=1 broadcast matmul after j0's matmuls (the j0
                        # epilogue below must see the write in program
                        # order); shares the ssq psum bank (freed by
                        # ssq_row's read)
                        bps = psum_ssq.tile([128, MC], FP32,
                                            name="psum_bc", bufs=1)
                        nc.tensor.matmul(bps[:], lhsT=ones_1[:],
                                         rhs=ssq_row[:],
                                         start=True, stop=True)
                        nc.scalar.activation(ssq_bc16[:], bps[:],
                                             AF.Identity)
                    # t_sq[p] = sum_f gram[p,f] * I[p,f]
                    tsq = tsq_pool.tile([128, 1], FP32, name="tsq")
                    scr = scr_pool.tile([128, 128], FP32, name="scr")
                    nc.vector.tensor_mul(scr[:], gram[:], ident[:])
                    nc.vector.reduce_sum(tsq[:], scr[:],
                                         axis=mybir.AxisListType.X)
                    # out = (psum + t_sq) + s_sq   (psum = -2*cross)
                    tmp = tmp_pool.tile([128, MC], FP16, name="tmp")
                    nc.scalar.activation(tmp[:], ps[:], AF.Identity,
                                         bias=tsq[:], scale=1.0)
                    nc.gpsimd.tensor_add(ob[:, jj, :], tmp[:], ssq_bc16[:])
                nc.sync.dma_start(out=out[g], in_=ob[:])

    nc.compile()
    return nc


_NC_CACHE = None


def _get_nc():
    global _NC_CACHE
    if _NC_CACHE is None:
        _NC_CACHE = build_nc()
    return _NC_CACHE


def stage_inputs(t_rep, s_rep, W, b):
    """Host-side layout + precision staging -> per-core input maps."""
    t_rep = np.asarray(t_rep, dtype=np.float32)
    s_rep = np.asarray(s_rep, dtype=np.float32)
    W = np.asarray(W, dtype=np.float32)
    b = np.asarray(b, dtype=np.float32)

    # t8[g, p, k, r] = t[g*512 + r, k*128 + p]
    t8 = np.ascontiguousarray(
        t_rep.reshape(NG, 512, KC, 128).transpose(0, 3, 2, 1)
    ).astype(NP_FP8)
    # w8[c, p, k, m] = W[c*384 + m, k*128 + p]
    w8 = np.ascontiguousarray(
        W.reshape(WCH, WCOLS, KC, 128).transpose(0, 3, 2, 1)
    ).astype(NP_FP8)
    # bneg2[p, k] = -2*b[k*128+p]
    bneg2 = np.ascontiguousarray((-2.0 * b).reshape(KC, 128).T)
    ident = np.eye(128, dtype=np.float32)

    in_maps = []
    for c in range(NCORES):
        s_slice = s_rep[c * MC:(c + 1) * MC]  # [512, D]
        # s8[p, k, r] = s_slice[r, k*128 + p]
        s8 = np.ascontiguousarray(
            s_slice.reshape(MC, KC, 128).transpose(2, 1, 0)
        ).astype(NP_FP8)
        in_maps.append({"t": t8, "s": s8, "w": w8, "bneg2": bneg2,
                        "ident": ident})
    return in_maps


def run_spmd(in_maps, **kwargs):
    nc = _get_nc()
    return run_bass_kernel_spmd(nc, in_maps, core_ids=list(range(NCORES)), **kwargs)


def gather_output(results):
    cols = []
    for c in range(NCORES):
        o = np.asarray(results[c]["out"])  # [NG, 128, 4, MC] fp16
        cols.append(o.transpose(0, 2, 1, 3).reshape(N, MC).astype(np.float32))
    return np.concatenate(cols, axis=1)


def kernel(t_rep, s_rep, W, b):
    in_maps = stage_inputs(t_rep, s_rep, W, b)
    res = run_spmd(in_maps)
    return gather_output(res.results)
